# revision 14
# baseline (speedup 1.0000x reference)
"""Trainium2 Bass kernel for nn_MixedLinearV2 (moe_routing).

y[b,s,o] = sum_i x[b,s,i] * (W[o,i]*coeff[o,i]) + b[o]*rowscale[o]
  coeff[o,i]  = sum_k weights[k] * row_mask[k,o] * col_mask[k,i]
  rowscale[o] = sum_k weights[k] * row_mask[k,o]

Strategy: data-parallel over batch (8 batch elements -> 8 NeuronCores).
Shard layout prep on host: x and W are laid out contraction-dim-major
(i on partitions) so they DMA straight into matmul operand tiles.

Per core (single fused pipeline, all phases interleaved):
  - W^T is DMAed straight into the SBUF-resident W_mixT tile and coeff^T
    (from the 9 mixing weights x masks, zero-padded to K=128) is
    multiplied in place chunk by chunk (DVE reads the coeff PSUM).
  - The mixed bias row is built on device and broadcast across partitions
    with a padded matmul.
  - Main loop: for each 128-row tile of x: 64 matmuls (8 k-tiles x 8
    512-wide out chunks, 4 PSUM banks active + 4 draining), DVE bias-add
    eviction, DMA out. The first few output-half passes are interleaved
    with the remaining coeff groups so the PE never waits on the W DMA.
Matmul dtype: float32r (reduced-precision fp32 path of the PE,
~1.6e-4 norm relative error vs the fp32 reference, 1 cycle/row).
"""

import sys
import types

import numpy as np

# ---- constants (hardcoded from the problem spec) ----
B, S, IN, OUT = 8, 4096, 1024, 4096
IN_DIMS = (512, 768, 1024)
OUT_MULTS = (2, 3, 4)
K9 = 9
P = 128
KT = IN // P          # 8 k-tiles
ST = S // P           # 32 s-tiles
OC = OUT // 512       # 8 out chunks of 512
N_CORES = 8

MAIN_DT_NAME = "f32r"  # one of: f32r, bf16, f32


def _ensure_ntff_hook():
    """Register the antenv.axon_hooks shim so trace=True can profile."""
    if 'antenv.axon_hooks' in sys.modules:
        return
    try:
        import antenv
    except ImportError:
        return
    mod = types.ModuleType('antenv.axon_hooks')
    mod._hook = None
    mod.set_axon_ntff_profile_hook = lambda h: setattr(mod, '_hook', h)
    mod.get_axon_ntff_profile_hook = lambda: mod._hook
    sys.modules['antenv.axon_hooks'] = mod
    antenv.axon_hooks = mod
    try:
        from trn_agent_boot.trn_boot import _ntff_profile_via_ctypes
        mod.set_axon_ntff_profile_hook(
            _ntff_profile_via_ctypes('/opt/axon/libaxon_pjrt.so'))
    except Exception:
        pass


def _masks_np():
    out_dims = np.array([m * i for i in IN_DIMS for m in OUT_MULTS])
    in_dims = np.array([i for i in IN_DIMS for _ in OUT_MULTS])
    row_mask = (np.arange(OUT)[None, :] < out_dims[:, None]).astype(np.float32)
    col_mask = (np.arange(IN)[None, :] < in_dims[:, None]).astype(np.float32)
    # zero-pad the 9 mask rows up to 128 partitions so the mask matmuls can
    # contract over a full 128-partition dim
    row_pad = np.zeros((P, OUT), np.float32)
    row_pad[:K9] = row_mask
    col_pad = np.zeros((P, IN), np.float32)
    col_pad[:K9] = col_mask
    return row_pad, col_pad


_BUILT = {}


def _build(main_dt_name=MAIN_DT_NAME):
    """Build + compile the SPMD Bass program (one program, 8 cores)."""
    if main_dt_name in _BUILT:
        return _BUILT[main_dt_name]

    import concourse.bacc as bacc
    import concourse.mybir as mybir
    from concourse.tile import TileContext

    F32 = mybir.dt.float32
    F32R = mybir.dt.float32r
    MAIN_DT = {"f32r": F32R, "bf16": mybir.dt.bfloat16, "f32": F32}[main_dt_name]
    IO_DT = F32R if main_dt_name == "f32r" else F32

    nc = bacc.Bacc("TRN2", target_bir_lowering=False, debug=False,
                   num_devices=N_CORES)

    # xT[s, p, it, q] = x[s*128+q, it*128+p] : k-major 128-row tiles
    xT_d = nc.declare_dram_parameter("xT", [ST, P, KT, P], IO_DT, isOutput=False)
    # WT[p, it, o] = W[o, it*128+p]
    wT_d = nc.declare_dram_parameter("WT", [P, KT, OUT], IO_DT, isOutput=False)
    b_d = nc.declare_dram_parameter("b", [1, OUT], F32, isOutput=False)
    wt9_d = nc.declare_dram_parameter("wts", [P, 1], F32R, isOutput=False)
    rowm_d = nc.declare_dram_parameter("rowm", [P, OUT], F32R, isOutput=False)
    colm_d = nc.declare_dram_parameter("colm", [P, IN], F32R, isOutput=False)
    bc0_d = nc.declare_dram_parameter("bc0", [P, P], F32, isOutput=False)
    y_d = nc.declare_dram_parameter("y", [S, OUT], F32, isOutput=True)

    with TileContext(nc) as tc:
        with (
            tc.tile_pool(name="persist", bufs=1) as persist,
            tc.tile_pool(name="xT_pool", bufs=6) as xT_pool,
            tc.tile_pool(name="ysb_pool", bufs=2) as ysb_pool,
            tc.tile_pool(name="ps_pool", bufs=8, space="PSUM") as ps_pool,
        ):
            wmixT = persist.tile([P, KT, OUT], MAIN_DT)   # [i_part, i_outer, o]
            bias_sb = persist.tile([P, OUT], F32)
            rowm = persist.tile([P, OUT], F32R)
            wcol = persist.tile([P, IN], F32R)
            wts = persist.tile([P, 1], F32R)
            bc0 = persist.tile([P, P], F32)

            # small consts first in the DMA queue so PE work starts early
            nc.sync.dma_start(wts[:], wt9_d[:])
            nc.sync.dma_start(wcol[:], colm_d[:])   # col mask; scaled in place
            nc.sync.dma_start(rowm[:], rowm_d[:])
            nc.sync.dma_start(bc0[:], bc0_d[:])
            nc.any.memzero(bias_sb[:])
            nc.sync.dma_start(bias_sb[0:1, :], b_d[:])

            # first x tiles interleaved into the 16MB W stream
            xT_tiles = {}

            def fetch_xT_early(s):
                xT_tiles[s] = xT_pool.tile([P, KT, P], MAIN_DT, tag="xT",
                                           name=f"xT_{s}")
                nc.sync.dma_start(xT_tiles[s][:], xT_d[s])

            fetch_xT_early(0)
            fetch_xT_early(1)
            for ocx in range(OC):
                nc.sync.dma_start(wmixT[:, :, ocx * 512:(ocx + 1) * 512],
                                  wT_d[:, :, ocx * 512:(ocx + 1) * 512])
                if ocx >= 3 and ocx <= 6:
                    fetch_xT_early(ocx - 1)

            # wcol[k,i] = weights[k] * col_mask[k,i] (in place)
            nc.vector.tensor_tensor(
                wcol[:], wcol[:], wts[:, 0:1].to_broadcast((P, IN)),
                mybir.AluOpType.mult)

            def bias_all():
                """All 16 bias matmuls upfront: rs pass, then broadcast pass."""
                for ocx in range(OC):
                    sl = slice(ocx * 512, (ocx + 1) * 512)
                    rs = ps_pool.tile([P, 512], F32, tag="ps", name=f"rs_{ocx}")
                    nc.tensor.matmul(rs[0:1, :], wts[:], rowm[:, sl],
                                     start=True, stop=True)
                    nc.vector.tensor_tensor(bias_sb[0:1, sl], bias_sb[0:1, sl],
                                            rs[0:1, :], mybir.AluOpType.mult)
                for ocx in range(OC):
                    sl = slice(ocx * 512, (ocx + 1) * 512)
                    bb = ps_pool.tile([P, 512], F32, tag="ps", name=f"bb_{ocx}")
                    nc.tensor.matmul(bb[:], bc0[:], bias_sb[:, sl],
                                     start=True, stop=True)
                    nc.vector.tensor_copy(bias_sb[:, sl], bb[:])

            def cf_group(ocx):
                """coeff^T chunk for all 8 k-tiles, multiplied into W_mixT."""
                sl = slice(ocx * 512, (ocx + 1) * 512)
                for it in range(KT):
                    cf = ps_pool.tile([P, 512], F32, tag="ps",
                                      name=f"cf_{ocx}_{it}")
                    nc.tensor.matmul(cf[:], wcol[:, it * P:(it + 1) * P],
                                     rowm[:, sl], start=True, stop=True)
                    nc.vector.tensor_tensor(
                        wmixT[:, it, sl], wmixT[:, it, sl], cf[:],
                        mybir.AluOpType.mult)

            def evict(s, half, yps):
                ysb = ysb_pool.tile([P, 2048], F32, tag="ysb",
                                    name=f"ysb_{s}_{half}")
                for j in range(4):
                    ocx = half * 4 + j
                    nc.vector.tensor_tensor(
                        ysb[:, j * 512:(j + 1) * 512], yps[j][:],
                        bias_sb[:, ocx * 512:(ocx + 1) * 512],
                        mybir.AluOpType.add)
                nc.sync.dma_start(
                    y_d[s * P:(s + 1) * P, half * 2048:(half + 1) * 2048],
                    ysb[:])

            def main_half_intro(s, half):
                """First visit of this half's 4 chunks: fuse cf groups in,
                bank-at-a-time so matmuls chase the W DMA chunk by chunk."""
                yps = []
                for j in range(4):
                    ocx = half * 4 + j
                    cf_group(ocx)
                    yp = ps_pool.tile([P, 512], F32, tag="ps",
                                      name=f"yps_{s}_{half}_{j}")
                    yps.append(yp)
                    for it in range(KT):
                        nc.tensor.matmul(
                            yp[:], xT_tiles[s][:, it, :],
                            wmixT[:, it, ocx * 512:(ocx + 1) * 512],
                            start=(it == 0), stop=(it == KT - 1))
                evict(s, half, yps)

            def main_half(s, half):
                yps = []
                for j in range(4):
                    ocx = half * 4 + j
                    yp = ps_pool.tile([P, 512], F32, tag="ps",
                                      name=f"yps_{s}_{half}_{j}")
                    yps.append(yp)
                    for it in range(KT):
                        nc.tensor.matmul(
                            yp[:], xT_tiles[s][:, it, :],
                            wmixT[:, it, ocx * 512:(ocx + 1) * 512],
                            start=(it == 0), stop=(it == KT - 1))
                evict(s, half, yps)

            def fetch_xT(s):
                if s < ST and s not in xT_tiles:
                    xT_tiles[s] = xT_pool.tile([P, KT, P], MAIN_DT, tag="xT",
                                               name=f"xT_{s}")
                    nc.sync.dma_start(xT_tiles[s][:], xT_d[s])

            bias_all()
            main_half_intro(0, 0)
            main_half(1, 0)
            main_half(2, 0)
            main_half(3, 0)
            main_half(4, 0)
            main_half(5, 0)
            main_half_intro(0, 1)
            fetch_xT(6)
            main_half(1, 1)
            fetch_xT(7)
            main_half(2, 1)
            fetch_xT(8)
            main_half(3, 1)
            fetch_xT(9)
            main_half(4, 1)
            main_half(5, 1)
            for s in range(6, ST):
                main_half(s, 0)
                fetch_xT(s + 4)
                main_half(s, 1)

    nc.compile()
    _BUILT[main_dt_name] = nc
    return nc


def _shard_layouts(inputs):
    """Host-side shard/layout prep: k-major tiles for x (per core) and W."""
    x = np.asarray(inputs["x"], np.float32)
    weights = np.asarray(inputs["weights"], np.float32)
    W = np.asarray(inputs["W"], np.float32)
    bias = np.asarray(inputs["b"], np.float32)

    row_pad, col_pad = _masks_np()
    wts_pad = np.zeros((P, 1), np.float32)
    wts_pad[:K9, 0] = weights
    bc0 = np.zeros((P, P), np.float32)
    bc0[0, :] = 1.0
    b_row = np.ascontiguousarray(bias[None, :])

    # WT[p, it, o] = W[o, it*128+p]
    WT = np.ascontiguousarray(W.reshape(OUT, KT, P).transpose(2, 1, 0))
    shared = {"WT": WT, "b": b_row, "wts": wts_pad, "rowm": row_pad,
              "colm": col_pad, "bc0": bc0}
    in_maps = []
    for c in range(N_CORES):
        # xT[s, p, it, q] = x[c, s*128+q, it*128+p]
        xT = np.ascontiguousarray(
            x[c].reshape(ST, P, KT, P).transpose(0, 3, 2, 1))
        in_maps.append(dict(shared, xT=xT))
    return in_maps


def _run(inputs, main_dt_name=MAIN_DT_NAME, trace=False, tmpdir=None):
    _ensure_ntff_hook()
    import concourse.bass_utils as bass_utils
    # artifact upload needs a bucket; keep traces local
    bass_utils.upload_artifacts = lambda tmpdir: f"local:{tmpdir}"
    from concourse.bass_utils import run_bass_kernel_spmd

    nc = _build(main_dt_name)
    in_maps = _shard_layouts(inputs)
    res = run_bass_kernel_spmd(nc, in_maps, core_ids=list(range(N_CORES)),
                               trace=trace, tmpdir=tmpdir)
    y = np.empty((B, S, OUT), np.float32)
    for c in range(N_CORES):
        y[c] = res.results[c]["y"]
    return y, res


def kernel(**inputs) -> np.ndarray:
    y, _ = _run(inputs, trace=False)
    return y


# revision 15
# speedup vs baseline: 1.1827x; 1.1827x over previous
"""Trainium2 Bass kernel for nn_MixedLinearV2 (moe_routing).

y[b,s,o] = sum_i x[b,s,i] * (W[o,i]*coeff[o,i]) + b[o]*rowscale[o]
  coeff[o,i]  = sum_k weights[k] * row_mask[k,o] * col_mask[k,i]
  rowscale[o] = sum_k weights[k] * row_mask[k,o]

Strategy: data-parallel over batch (8 batch elements -> 8 NeuronCores).
Shard layout prep on host: x and W are laid out contraction-dim-major
(i on partitions) so they DMA straight into matmul operand tiles.

Per core (single fused pipeline, all phases interleaved):
  - W^T is DMAed straight into the SBUF-resident W_mixT tile and coeff^T
    (from the 9 mixing weights x masks, zero-padded to K=128) is
    multiplied in place chunk by chunk (DVE reads the coeff PSUM).
  - The mixed bias row is built on device and broadcast across partitions
    with a padded matmul.
  - Main loop: for each 128-row tile of x: 64 matmuls (8 k-tiles x 8
    512-wide out chunks, 4 PSUM banks active + 4 draining), DVE bias-add
    eviction, DMA out. The first few output-half passes are interleaved
    with the remaining coeff groups so the PE never waits on the W DMA.
Matmul dtype: float32r (reduced-precision fp32 path of the PE,
~1.6e-4 norm relative error vs the fp32 reference, 1 cycle/row).
"""

import sys
import types

import numpy as np

# ---- constants (hardcoded from the problem spec) ----
B, S, IN, OUT = 8, 4096, 1024, 4096
IN_DIMS = (512, 768, 1024)
OUT_MULTS = (2, 3, 4)
K9 = 9
P = 128
KT = IN // P          # 8 k-tiles
ST = S // P           # 32 s-tiles
OC = OUT // 512       # 8 out chunks of 512
N_CORES = 8

MAIN_DT_NAME = "f32r"  # one of: f32r, bf16, f32


def _ensure_ntff_hook():
    """Register the antenv.axon_hooks shim so trace=True can profile."""
    if 'antenv.axon_hooks' in sys.modules:
        return
    try:
        import antenv
    except ImportError:
        return
    mod = types.ModuleType('antenv.axon_hooks')
    mod._hook = None
    mod.set_axon_ntff_profile_hook = lambda h: setattr(mod, '_hook', h)
    mod.get_axon_ntff_profile_hook = lambda: mod._hook
    sys.modules['antenv.axon_hooks'] = mod
    antenv.axon_hooks = mod
    try:
        from trn_agent_boot.trn_boot import _ntff_profile_via_ctypes
        mod.set_axon_ntff_profile_hook(
            _ntff_profile_via_ctypes('/opt/axon/libaxon_pjrt.so'))
    except Exception:
        pass


def _masks_np():
    out_dims = np.array([m * i for i in IN_DIMS for m in OUT_MULTS])
    in_dims = np.array([i for i in IN_DIMS for _ in OUT_MULTS])
    row_mask = (np.arange(OUT)[None, :] < out_dims[:, None]).astype(np.float32)
    col_mask = (np.arange(IN)[None, :] < in_dims[:, None]).astype(np.float32)
    # zero-pad the 9 mask rows up to 128 partitions so the mask matmuls can
    # contract over a full 128-partition dim
    row_pad = np.zeros((P, OUT), np.float32)
    row_pad[:K9] = row_mask
    col_pad = np.zeros((P, IN), np.float32)
    col_pad[:K9] = col_mask
    return row_pad, col_pad


_BUILT = {}


def _build(main_dt_name=MAIN_DT_NAME):
    """Build + compile the SPMD Bass program (one program, 8 cores)."""
    if main_dt_name in _BUILT:
        return _BUILT[main_dt_name]

    import concourse.bacc as bacc
    import concourse.mybir as mybir
    from concourse.tile import TileContext

    F32 = mybir.dt.float32
    F32R = mybir.dt.float32r
    MAIN_DT = {"f32r": F32R, "bf16": mybir.dt.bfloat16, "f32": F32}[main_dt_name]
    IO_DT = F32R if main_dt_name == "f32r" else F32

    nc = bacc.Bacc("TRN2", target_bir_lowering=False, debug=False,
                   num_devices=N_CORES)

    # xT[s, p, it, q] = x[s*128+q, it*128+p] : k-major 128-row tiles
    xT_d = nc.declare_dram_parameter("xT", [ST, P, KT, P], IO_DT, isOutput=False)
    # WT[p, it, o] = W[o, it*128+p]
    wT_d = nc.declare_dram_parameter("WT", [P, KT, OUT], IO_DT, isOutput=False)
    b_d = nc.declare_dram_parameter("b", [1, OUT], F32, isOutput=False)
    wt9_d = nc.declare_dram_parameter("wts", [P, 1], F32R, isOutput=False)
    rowm_d = nc.declare_dram_parameter("rowm", [P, OUT], F32R, isOutput=False)
    colm_d = nc.declare_dram_parameter("colm", [P, IN], F32R, isOutput=False)
    bc0_d = nc.declare_dram_parameter("bc0", [P, P], F32, isOutput=False)
    y_d = nc.declare_dram_parameter("y", [S, OUT], F32, isOutput=True)

    with TileContext(nc) as tc:
        with (
            tc.tile_pool(name="persist", bufs=1) as persist,
            tc.tile_pool(name="xT_pool", bufs=6) as xT_pool,
            tc.tile_pool(name="ysb_pool", bufs=2) as ysb_pool,
            tc.tile_pool(name="ps_pool", bufs=8, space="PSUM") as ps_pool,
        ):
            wmixT = persist.tile([P, KT, OUT], MAIN_DT)   # [i_part, i_outer, o]
            bias_sb = persist.tile([P, OUT], F32)
            rowm0 = persist.tile([P, OUT // 2], F32R)
            rowm1 = persist.tile([P, OUT // 2], F32R)

            def rowm_sl(ocx):
                t = rowm0 if ocx < 4 else rowm1
                off = (ocx % 4) * 512
                return t[:, off:off + 512]
            wcol = persist.tile([P, IN], F32R)
            wts = persist.tile([P, 1], F32R)
            bc0 = persist.tile([P, P], F32)

            # small consts first in the DMA queue so PE work starts early
            nc.sync.dma_start(wts[:], wt9_d[:])
            nc.sync.dma_start(wcol[:], colm_d[:])   # col mask; scaled in place
            nc.sync.dma_start(rowm0[:], rowm_d[:, :OUT // 2])
            nc.sync.dma_start(rowm1[:], rowm_d[:, OUT // 2:])
            nc.sync.dma_start(bc0[:], bc0_d[:])
            nc.any.memzero(bias_sb[:])
            nc.sync.dma_start(bias_sb[0:1, :], b_d[:])

            # first x tiles interleaved into the 16MB W stream
            xT_tiles = {}

            def fetch_xT_early(s):
                xT_tiles[s] = xT_pool.tile([P, KT, P], MAIN_DT, tag="xT",
                                           name=f"xT_{s}")
                nc.sync.dma_start(xT_tiles[s][:], xT_d[s])

            fetch_xT_early(0)
            fetch_xT_early(1)
            for ocx in range(OC):
                nc.sync.dma_start(wmixT[:, :, ocx * 512:(ocx + 1) * 512],
                                  wT_d[:, :, ocx * 512:(ocx + 1) * 512])
                if ocx >= 3 and ocx <= 6:
                    fetch_xT_early(ocx - 1)

            # wcol[k,i] = weights[k] * col_mask[k,i] (in place)
            nc.vector.tensor_tensor(
                wcol[:], wcol[:], wts[:, 0:1].to_broadcast((P, IN)),
                mybir.AluOpType.mult)

            def bias_all():
                """All 16 bias matmuls upfront: rs pass, then broadcast pass."""
                for ocx in range(OC):
                    sl = slice(ocx * 512, (ocx + 1) * 512)
                    rs = ps_pool.tile([P, 512], F32, tag="ps", name=f"rs_{ocx}")
                    nc.tensor.matmul(rs[0:1, :], wts[:], rowm_sl(ocx),
                                     start=True, stop=True)
                    nc.vector.tensor_tensor(bias_sb[0:1, sl], bias_sb[0:1, sl],
                                            rs[0:1, :], mybir.AluOpType.mult)
                for ocx in range(OC):
                    sl = slice(ocx * 512, (ocx + 1) * 512)
                    bb = ps_pool.tile([P, 512], F32, tag="ps", name=f"bb_{ocx}")
                    nc.tensor.matmul(bb[:], bc0[:], bias_sb[:, sl],
                                     start=True, stop=True)
                    nc.vector.tensor_copy(bias_sb[:, sl], bb[:])

            def cf_group(ocx):
                """coeff^T chunk for all 8 k-tiles, multiplied into W_mixT."""
                sl = slice(ocx * 512, (ocx + 1) * 512)
                for it in range(KT):
                    cf = ps_pool.tile([P, 512], F32, tag="ps",
                                      name=f"cf_{ocx}_{it}")
                    nc.tensor.matmul(cf[:], wcol[:, it * P:(it + 1) * P],
                                     rowm_sl(ocx), start=True, stop=True)
                    nc.vector.tensor_tensor(
                        wmixT[:, it, sl], wmixT[:, it, sl], cf[:],
                        mybir.AluOpType.mult)

            def evict(s, half, yps):
                ysb = ysb_pool.tile([P, 2048], F32, tag="ysb",
                                    name=f"ysb_{s}_{half}")
                for j in range(4):
                    ocx = half * 4 + j
                    nc.vector.tensor_tensor(
                        ysb[:, j * 512:(j + 1) * 512], yps[j][:],
                        bias_sb[:, ocx * 512:(ocx + 1) * 512],
                        mybir.AluOpType.add)
                nc.sync.dma_start(
                    y_d[s * P:(s + 1) * P, half * 2048:(half + 1) * 2048],
                    ysb[:])

            def main_half_intro(s, half):
                """First visit of this half's 4 chunks: fuse cf groups in,
                bank-at-a-time so matmuls chase the W DMA chunk by chunk."""
                yps = []
                for j in range(4):
                    ocx = half * 4 + j
                    cf_group(ocx)
                    yp = ps_pool.tile([P, 512], F32, tag="ps",
                                      name=f"yps_{s}_{half}_{j}")
                    yps.append(yp)
                    for it in range(KT):
                        nc.tensor.matmul(
                            yp[:], xT_tiles[s][:, it, :],
                            wmixT[:, it, ocx * 512:(ocx + 1) * 512],
                            start=(it == 0), stop=(it == KT - 1))
                evict(s, half, yps)

            def main_half(s, half):
                yps = []
                for j in range(4):
                    ocx = half * 4 + j
                    yp = ps_pool.tile([P, 512], F32, tag="ps",
                                      name=f"yps_{s}_{half}_{j}")
                    yps.append(yp)
                    for it in range(KT):
                        nc.tensor.matmul(
                            yp[:], xT_tiles[s][:, it, :],
                            wmixT[:, it, ocx * 512:(ocx + 1) * 512],
                            start=(it == 0), stop=(it == KT - 1))
                evict(s, half, yps)

            def fetch_xT(s):
                if s < ST and s not in xT_tiles:
                    xT_tiles[s] = xT_pool.tile([P, KT, P], MAIN_DT, tag="xT",
                                               name=f"xT_{s}")
                    nc.sync.dma_start(xT_tiles[s][:], xT_d[s])

            bias_all()
            main_half_intro(0, 0)
            main_half(1, 0)
            cf_group(4)
            main_half(2, 0)
            cf_group(5)
            main_half(3, 0)
            cf_group(6)
            main_half(4, 0)
            cf_group(7)
            main_half(5, 0)
            main_half(0, 1)
            fetch_xT(6)
            main_half(1, 1)
            fetch_xT(7)
            main_half(2, 1)
            fetch_xT(8)
            main_half(3, 1)
            fetch_xT(9)
            main_half(4, 1)
            main_half(5, 1)
            for s in range(6, ST):
                main_half(s, 0)
                fetch_xT(s + 4)
                main_half(s, 1)

    nc.compile()
    _BUILT[main_dt_name] = nc
    return nc


def _shard_layouts(inputs):
    """Host-side shard/layout prep: k-major tiles for x (per core) and W."""
    x = np.asarray(inputs["x"], np.float32)
    weights = np.asarray(inputs["weights"], np.float32)
    W = np.asarray(inputs["W"], np.float32)
    bias = np.asarray(inputs["b"], np.float32)

    row_pad, col_pad = _masks_np()
    wts_pad = np.zeros((P, 1), np.float32)
    wts_pad[:K9, 0] = weights
    bc0 = np.zeros((P, P), np.float32)
    bc0[0, :] = 1.0
    b_row = np.ascontiguousarray(bias[None, :])

    # WT[p, it, o] = W[o, it*128+p]
    WT = np.ascontiguousarray(W.reshape(OUT, KT, P).transpose(2, 1, 0))
    shared = {"WT": WT, "b": b_row, "wts": wts_pad, "rowm": row_pad,
              "colm": col_pad, "bc0": bc0}
    in_maps = []
    for c in range(N_CORES):
        # xT[s, p, it, q] = x[c, s*128+q, it*128+p]
        xT = np.ascontiguousarray(
            x[c].reshape(ST, P, KT, P).transpose(0, 3, 2, 1))
        in_maps.append(dict(shared, xT=xT))
    return in_maps


def _run(inputs, main_dt_name=MAIN_DT_NAME, trace=False, tmpdir=None):
    _ensure_ntff_hook()
    import concourse.bass_utils as bass_utils
    # artifact upload needs a bucket; keep traces local
    bass_utils.upload_artifacts = lambda tmpdir: f"local:{tmpdir}"
    from concourse.bass_utils import run_bass_kernel_spmd

    nc = _build(main_dt_name)
    in_maps = _shard_layouts(inputs)
    res = run_bass_kernel_spmd(nc, in_maps, core_ids=list(range(N_CORES)),
                               trace=trace, tmpdir=tmpdir)
    y = np.empty((B, S, OUT), np.float32)
    for c in range(N_CORES):
        y[c] = res.results[c]["y"]
    return y, res


def kernel(**inputs) -> np.ndarray:
    y, _ = _run(inputs, trace=False)
    return y
